# revision 2
# baseline (speedup 1.0000x reference)
"""MatchLSTM Trainium2 kernel v2: data-parallel over batch (8 cores, 1 elem each).

Column-form recurrences: hidden state lives as SBUF columns (chunk0 [128,1],
chunk1 [22,1]+bias-lane), every per-step matmul outputs a [<=128,1] PSUM
column (lhsT = weight slice, rhs = hidden/embedding column), and the gate
elementwise tail uses fused tensor_scalar ops on per-partition columns.
No per-step transposes. The ctx-GRU, q-GRU and match loops are independent
dep-chains that the Tile scheduler overlaps.
"""
import math
from contextlib import ExitStack

import numpy as np
import ml_dtypes

import concourse.bacc as bacc
import concourse.bass as bass
import concourse.mybir as mybir
import concourse.tile as tile
from concourse.bass_utils import run_bass_kernel_spmd

F32 = mybir.dt.float32
BF16 = mybir.dt.bfloat16
I32 = mybir.dt.int32
AF = mybir.ActivationFunctionType
OP = mybir.AluOpType
BF = ml_dtypes.bfloat16

H = 150
D = 300
J = 64
V = 100000

# gate-column layout inside the per-GRU psum tile:
#   col 0: r0 (gates   0:128), col 1: z0 (150:278),
#   col 2: r1 (128:150),       col 3: z1 (278:300),
#   col 4: hn0 (300:428),      col 5: hn1 (428:450),
#   col 6: xn0 (300:428 x-part), col 7: xn1 (428:450 x-part)
RZ_COLS = [(0, 0, 128), (1, 150, 278), (2, 128, 150), (3, 278, 300)]
HN_COLS = [(4, 300, 428), (5, 428, 450)]
XN_COLS = [(6, 300, 428), (7, 428, 450)]


def build(T=400, dbg=False):
    NT = math.ceil(T / 128)
    tsz = [min(128, T - 128 * g) for g in range(NT)]

    nc = bacc.Bacc("TRN2", target_bir_lowering=False, debug=False, num_devices=8)

    dram = {}

    def din(name, shape, dt):
        dram[name] = nc.dram_tensor(name, list(shape), dt, kind="ExternalInput")
        return dram[name]

    E_d = din("E", [V, D], F32)
    din("ctx_idx", [128, NT], I32)
    din("q_idx", [J, 1], I32)
    din("Ifp", [128, 128], F32)
    din("Ibf", [128, 128], BF16)
    din("initrow", [1, 1024], BF16)  # 0,1,0,1,... for aug bias lanes
    din("onesr", [1, 512], BF16)
    din("bihm", [1, 450], BF16)
    din("wcol", [128, 2], BF16)
    wnames = []
    for g in ("q", "c"):
        wnames += [(f"WihT_{g}_0", (128, 450)), (f"WihT_{g}_1", (128, 450)),
                   (f"WihT_{g}_2", (45, 450))]
    for g in ("q", "c", "m"):
        wnames += [(f"WhhT_{g}_0", (128, 450)), (f"WhhT_{g}_1", (23, 450))]
    wnames += [("WcT_0", (128, 450)), ("WcT_1", (22, 450)),
               ("W2T_0", (128, 450)), ("W2T_1", (22, 450)),
               ("Wp_0", (128, H)), ("Wp_1", (22, H)),
               ("Wr_0", (128, H)), ("Wr_1", (22, H)),
               ("Wq_0", (128, H)), ("Wq_1", (22, H))]
    for n, s in wnames:
        din(n, s, BF16)
    hr_d = nc.dram_tensor("hr", [T + 1, H], F32, kind="ExternalOutput")
    if dbg:
        dbg_d = {n: nc.dram_tensor(n, s, BF16, kind="ExternalOutput")
                 for n, s in (("hq_dbg", [128, 2 * (J + 1)]),
                              ("hc_dbg", [128, 2 * (T + 1)]),
                              ("hm_dbg", [128, 2 * (T + 1)]),
                              ("ecT0_dbg", [128, T]),
                              ("gt0_dbg", [128, J]),
                              ("gt1_dbg", [22, J]),
                              ("hqw2_dbg", [J + 1, 450]),
                              ("whqT0_dbg", [128, J]),
                              ("attn_dbg", [J + 1, 1]))}
        dbg_f = {n: nc.dram_tensor(n, s, F32, kind="ExternalOutput")
                 for n, s in (("sm_dbg", [128, 4]), ("xnm_dbg", [128, 2]),
                              ("nnm_dbg", [128, 2]), ("ddm_dbg", [128, 2]),
                              ("usb_dbg", [128, 2]), ("pm_dbg", [128, 12]))}

    with tile.TileContext(nc) as tc, ExitStack() as st:
        sb = st.enter_context(tc.tile_pool(name="sb", bufs=1))

        def sbt(name, shape, dt):
            return sb.tile(list(shape), dt, tag=name, name=name)

        W = {n: sbt(n, s, BF16) for n, s in wnames}
        Ifp = sbt("Ifp", (128, 128), F32)
        Ibf = sbt("Ibf", (128, 128), BF16)
        bihm = sbt("bihm", (1, 450), BF16)
        wcol = sbt("wcol", (128, 2), BF16)
        cidx = sbt("cidx", (128, NT), I32)
        qidx = sbt("qidx", (J, 1), I32)
        ec = [sbt(f"ec{g}", (128, D), F32) for g in range(NT)]
        eq = sbt("eq", (J, D), F32)
        ecT = [sbt("ecT0", (128, T), BF16), sbt("ecT1", (128, T), BF16),
               sbt("ecT2", (45, T), BF16)]
        eqT = [sbt("eqT0", (128, J), BF16), sbt("eqT1", (128, J), BF16),
               sbt("eqT2", (45, J), BF16)]
        HqAB = sbt("HqAB", (128, 2 * (J + 1)), BF16)
        HcAB = sbt("HcAB", (128, 2 * (T + 1)), BF16)
        HmAB = sbt("HmAB", (128, 2 * (T + 1)), BF16)
        whqT0 = sbt("whqT0", (128, J), BF16)
        whqT1 = sbt("whqT1", (22, J), BF16)
        HqW2 = sbt("HqW2", (J + 1, 450), BF16)
        attn_aug = sbt("attn_aug", (J + 1, 1), BF16)
        GT0 = sbt("GT0", (128, J), BF16)
        GT1 = sbt("GT1", (22, J), BF16)
        usb = sbt("usb", (128, 2), F32)
        S = {g: sbt(f"S_{g}", (128, 4), F32) for g in ("q", "c", "m")}
        XN = {g: sbt(f"XN_{g}", (128, 2), F32) for g in ("q", "c", "m")}
        NNt = {g: sbt(f"NN_{g}", (128, 2), F32) for g in ("q", "c", "m")}
        DDt = {g: sbt(f"DD_{g}", (128, 2), F32) for g in ("q", "c", "m")}
        OutR = sbt("OutR", (128, 152), F32)

        # ---- load inputs ----
        for n, _ in wnames:
            nc.sync.dma_start(W[n][:], dram[n].ap())
        nc.sync.dma_start(Ifp[:], dram["Ifp"].ap())
        nc.sync.dma_start(Ibf[:], dram["Ibf"].ap())
        nc.sync.dma_start(bihm[:], dram["bihm"].ap())
        nc.sync.dma_start(wcol[:], dram["wcol"].ap())
        nc.sync.dma_start(cidx[:], dram["ctx_idx"].ap())
        nc.sync.dma_start(qidx[:], dram["q_idx"].ap())

        # ---- init state: h0 = 0; aug bias lane (row 22 of odd cols) = 1.0 ----
        for hab, ncols in ((HqAB, 2 * (J + 1)), (HcAB, 2 * (T + 1)),
                           (HmAB, 2 * (T + 1))):
            nc.vector.memset(hab[:, 0:2], 0.0)
            nc.sync.dma_start(hab[22:23, 0:ncols],
                             dram["initrow"].ap()[0:1, 0:ncols])
        nc.vector.memset(attn_aug[0:J + 1, 0:1], 0.0)
        nc.vector.memset(attn_aug[J:J + 1, 0:1], 1.0)
        nc.sync.dma_start(ecT[2][44:45, 0:T], dram["onesr"].ap()[0:1, 0:T])
        nc.sync.dma_start(eqT[2][44:45, 0:J], dram["onesr"].ap()[0:1, 0:J])
        nc.sync.dma_start(HqW2[J:J + 1, :], dram["bihm"].ap())

        # ---- embedding gathers ----
        for g in range(NT):
            nc.gpsimd.indirect_dma_start(
                out=ec[g][:], out_offset=None, in_=E_d.ap(),
                in_offset=bass.IndirectOffsetOnAxis(ap=cidx[:, g:g + 1], axis=0))
        nc.gpsimd.indirect_dma_start(
            out=eq[:], out_offset=None, in_=E_d.ap(),
            in_offset=bass.IndirectOffsetOnAxis(ap=qidx[:, 0:1], axis=0))

        dch = [(0, 128), (128, 128), (256, 44)]

        # ---- embedding transposes: ec/eq -> ecT/eqT (bf16 columns) ----
        with tc.tile_pool(name="pre_ps", bufs=2, space="PSUM") as pps:
            for g in range(NT):
                toff = 128 * g
                for k, (doff, dsz) in enumerate(dch):
                    tp = pps.tile([128, 128], F32, tag="tp", name="tp")
                    nc.tensor.transpose(tp[0:dsz, 0:tsz[g]],
                                        ec[g][0:tsz[g], doff:doff + dsz],
                                        Ifp[0:tsz[g], 0:tsz[g]])
                    nc.vector.tensor_copy(ecT[k][0:dsz, toff:toff + tsz[g]],
                                          tp[0:dsz, 0:tsz[g]])
            for k, (doff, dsz) in enumerate(dch):
                tp = pps.tile([128, 128], F32, tag="tp", name="tp")
                nc.tensor.transpose(tp[0:dsz, 0:J], eq[0:J, doff:doff + dsz],
                                    Ifp[0:J, 0:J])
                nc.vector.tensor_copy(eqT[k][0:dsz, 0:J], tp[0:dsz, 0:J])

        # ---- persistent psum pools (one bank-sized tile per GRU chain) ----
        # columns: 0:8 gate psum P, 8:10 narg, 10:12 nn, 12:14 u, 14:15 attn
        psA = st.enter_context(tc.tile_pool(name="psA", bufs=1, space="PSUM"))
        P = {g: psA.tile([128, 12], F32, tag=f"PS_{g}", name=f"PS_{g}")
             for g in ("q", "c", "m")}
        CU, CA = 8, 10  # u / attn column offsets in the match PS tile

        def gru_tail(g, Pt, hprev_pair, hout0, hout1):
            """Gate elementwise tail shared by all three GRUs.

            hprev_pair: [128,2] AP of the previous hidden column pair;
            hout*: APs for the new hidden column chunks."""
            Sg, XNg, NNg, DDg = S[g], XN[g], NNt[g], DDt[g]
            # xn (x-part of n gate) psum -> sbuf (one [128,2] copy; junk rows ok)
            nc.vector.tensor_copy(XNg[0:128, 0:2], Pt[0:128, 6:8])
            # sigmoid over r0,z0,r1,z1 in one shot (junk rows of cols 2:4 unused)
            nc.scalar.activation(Sg[0:128, 0:4], Pt[0:128, 0:4], AF.Sigmoid)
            # n = tanh(r * hn + xn) fused into one activation per chunk
            nc.scalar.activation(NNg[0:128, 0:1], Pt[0:128, 4:5], AF.Tanh,
                                 bias=XNg[0:128, 0:1], scale=Sg[0:128, 0:1])
            nc.scalar.activation(NNg[0:22, 1:2], Pt[0:22, 5:6], AF.Tanh,
                                 bias=XNg[0:22, 1:2], scale=Sg[0:22, 2:3])
            # dd = h - n  (both chunks in one op; junk rows unused)
            nc.gpsimd.tensor_tensor(out=DDg[0:128, 0:2], in0=hprev_pair,
                                    in1=NNg[0:128, 0:2], op=OP.subtract)
            # h2 = dd * z + n
            nc.gpsimd.tensor_scalar(
                out=hout0, in0=DDg[0:128, 0:1], scalar1=Sg[0:128, 1:2],
                scalar2=NNg[0:128, 0:1], op0=OP.mult, op1=OP.add)
            nc.vector.tensor_scalar(
                out=hout1, in0=DDg[0:22, 1:2], scalar1=Sg[0:22, 3:4],
                scalar2=NNg[0:22, 1:2], op0=OP.mult, op1=OP.add)

        def enc_step(g, t, HAB, xT, xsz):
            """One encoder GRU step: x-projection + h-projection column mms,
            then the gate tail. xT = [ecT|eqT], xsz = per-chunk K sizes."""
            Pt = P[g]
            w0, w1 = W[f"WhhT_{g}_0"], W[f"WhhT_{g}_1"]
            x0, x1, x2 = (W[f"WihT_{g}_0"], W[f"WihT_{g}_1"],
                          W[f"WihT_{g}_2"])
            h0 = HAB[:, 2 * t:2 * t + 1]
            h1 = HAB[0:23, 2 * t + 1:2 * t + 2]
            # per-column consecutive accumulation groups: x k-chunks + h-proj
            for col, m0, m1 in RZ_COLS + XN_COLS:
                msz = m1 - m0
                for k, (xw, ksz) in enumerate(((x0, xsz[0]), (x1, xsz[1]),
                                               (x2, xsz[2]))):
                    last = (k == 2) and col in (6, 7)
                    nc.tensor.matmul(Pt[0:msz, col:col + 1],
                                     xw[0:ksz, m0:m1],
                                     xT[k][0:ksz, t:t + 1],
                                     start=(k == 0), stop=last)
                if col not in (6, 7):
                    nc.tensor.matmul(Pt[0:msz, col:col + 1], w0[:, m0:m1], h0,
                                     start=False, stop=False)
                    nc.tensor.matmul(Pt[0:msz, col:col + 1], w1[0:23, m0:m1],
                                     h1, start=False, stop=True)
            for col, m0, m1 in HN_COLS:
                msz = m1 - m0
                nc.tensor.matmul(Pt[0:msz, col:col + 1], w0[:, m0:m1], h0,
                                 start=True, stop=False)
                nc.tensor.matmul(Pt[0:msz, col:col + 1], w1[0:23, m0:m1], h1,
                                 start=False, stop=True)
            gru_tail(g, Pt, HAB[0:128, 2 * t:2 * t + 2],
                     HAB[0:128, 2 * t + 2:2 * t + 3],
                     HAB[0:22, 2 * t + 3:2 * t + 4])

        def match_step(t):
            """One match step: u = Wr@hm + Wp@hc; G = tanh(whqT + u);
            attn = G^T w; gates = Wc@hc + Whh@hm + HqW2^T@[attn;1]."""
            Pt = P["m"]
            hm0 = HmAB[:, 2 * t:2 * t + 1]
            hm1 = HmAB[0:23, 2 * t + 1:2 * t + 2]
            hm1s = HmAB[0:22, 2 * t + 1:2 * t + 2]
            hc0 = HcAB[:, 2 * t + 2:2 * t + 3]
            hc1 = HcAB[0:22, 2 * t + 3:2 * t + 4]
            # u columns
            for ci, (m0, m1) in enumerate(((0, 128), (128, 150))):
                msz = m1 - m0
                uc = slice(CU + ci, CU + ci + 1)
                nc.tensor.matmul(Pt[0:msz, uc], W["Wr_0"][:, m0:m1], hm0,
                                 start=True, stop=False)
                nc.tensor.matmul(Pt[0:msz, uc], W["Wr_1"][0:22, m0:m1],
                                 hm1s, start=False, stop=False)
                nc.tensor.matmul(Pt[0:msz, uc], W["Wp_0"][:, m0:m1], hc0,
                                 start=False, stop=False)
                nc.tensor.matmul(Pt[0:msz, uc], W["Wp_1"][0:22, m0:m1],
                                 hc1, start=False, stop=True)
            nc.vector.tensor_copy(usb[0:128, 0:2], Pt[0:128, CU:CU + 2])
            # G^T = tanh(whqT + u) in two partition chunks
            nc.scalar.activation(GT0[0:128, 0:J], whqT0[0:128, 0:J], AF.Tanh,
                                 bias=usb[0:128, 0:1])
            nc.scalar.activation(GT1[0:22, 0:J], whqT1[0:22, 0:J], AF.Tanh,
                                 bias=usb[0:22, 1:2])
            # attn column = GT^T w
            nc.tensor.matmul(Pt[0:J, CA:CA + 1], GT0[0:128, 0:J],
                             wcol[0:128, 0:1], start=True, stop=False)
            nc.tensor.matmul(Pt[0:J, CA:CA + 1], GT1[0:22, 0:J],
                             wcol[0:22, 1:2], start=False, stop=True)
            nc.vector.tensor_copy(attn_aug[0:J, 0:1], Pt[0:J, CA:CA + 1])
            # gate columns: zx (Wc@hc) + Whh@hm + HqW2^T@[attn;1]
            # each column's accumulation group is emitted consecutively
            for col, m0, m1 in RZ_COLS + XN_COLS:
                msz = m1 - m0
                nc.tensor.matmul(Pt[0:msz, col:col + 1], W["WcT_0"][:, m0:m1],
                                 hc0, start=True, stop=False)
                nc.tensor.matmul(Pt[0:msz, col:col + 1],
                                 W["WcT_1"][0:22, m0:m1], hc1,
                                 start=False, stop=False)
                if col not in (6, 7):
                    nc.tensor.matmul(Pt[0:msz, col:col + 1],
                                     W["WhhT_m_0"][:, m0:m1], hm0,
                                     start=False, stop=False)
                    nc.tensor.matmul(Pt[0:msz, col:col + 1],
                                     W["WhhT_m_1"][0:23, m0:m1], hm1,
                                     start=False, stop=False)
                nc.tensor.matmul(Pt[0:msz, col:col + 1],
                                 HqW2[0:J + 1, m0:m1], attn_aug[0:J + 1, 0:1],
                                 start=False, stop=True)
            for col, m0, m1 in HN_COLS:
                msz = m1 - m0
                nc.tensor.matmul(Pt[0:msz, col:col + 1],
                                 W["WhhT_m_0"][:, m0:m1], hm0,
                                 start=True, stop=False)
                nc.tensor.matmul(Pt[0:msz, col:col + 1],
                                 W["WhhT_m_1"][0:23, m0:m1], hm1,
                                 start=False, stop=True)
            gru_tail("m", Pt, HmAB[0:128, 2 * t:2 * t + 2],
                     HmAB[0:128, 2 * t + 2:2 * t + 3],
                     HmAB[0:22, 2 * t + 3:2 * t + 4])

        # ---- q-GRU ----
        for j in range(J):
            enc_step("q", j, HqAB, eqT, (128, 128, 45))
        # ---- whqT + HqW2 prep ----
        hq_c0 = HqAB[0:128, 2:2 * (J + 1):2]
        hq_c1 = HqAB[0:22, 3:2 * (J + 1):2]
        with tc.tile_pool(name="prep_ps", bufs=1, space="PSUM") as qps:
            wq_ps = qps.tile([128, J], F32, tag="wq", name="wq")
            hw_ps = qps.tile([J, 450], F32, tag="hw", name="hw")
            for ci, (m0, m1) in enumerate(((0, 128), (128, 150))):
                msz = m1 - m0
                nc.tensor.matmul(wq_ps[0:msz, 0:J], W["Wq_0"][:, m0:m1], hq_c0,
                                 start=True, stop=False)
                nc.tensor.matmul(wq_ps[0:msz, 0:J], W["Wq_1"][0:22, m0:m1],
                                 hq_c1, start=False, stop=True)
                dst = whqT0 if ci == 0 else whqT1
                nc.vector.tensor_copy(dst[0:msz, 0:J], wq_ps[0:msz, 0:J])
            nc.tensor.matmul(hw_ps[0:J, :], hq_c0, W["W2T_0"][:, :],
                             start=True, stop=False)
            nc.tensor.matmul(hw_ps[0:J, :], hq_c1, W["W2T_1"][0:22, :],
                             start=False, stop=True)
            nc.vector.tensor_copy(HqW2[0:J, :], hw_ps[0:J, :])

        # ---- ctx-GRU + match loop (scheduler overlaps the chains) ----
        for t in range(T):
            enc_step("c", t, HcAB, ecT, (128, 128, 45))
            match_step(t)

        if dbg:
            nc.sync.dma_start(dbg_d["hq_dbg"].ap(), HqAB[:])
            nc.sync.dma_start(dbg_d["hc_dbg"].ap(), HcAB[:])
            nc.sync.dma_start(dbg_d["hm_dbg"].ap(), HmAB[:])
            nc.sync.dma_start(dbg_d["ecT0_dbg"].ap(), ecT[0][:])
            nc.sync.dma_start(dbg_d["gt0_dbg"].ap(), GT0[:])
            nc.sync.dma_start(dbg_d["gt1_dbg"].ap(), GT1[:])
            nc.sync.dma_start(dbg_d["hqw2_dbg"].ap(), HqW2[:])
            nc.sync.dma_start(dbg_d["whqT0_dbg"].ap(), whqT0[:])
            nc.sync.dma_start(dbg_d["attn_dbg"].ap(), attn_aug[:])
            nc.sync.dma_start(dbg_f["sm_dbg"].ap(), S["m"][:])
            nc.sync.dma_start(dbg_f["xnm_dbg"].ap(), XN["m"][:])
            nc.sync.dma_start(dbg_f["nnm_dbg"].ap(), NNt["m"][:])
            nc.sync.dma_start(dbg_f["ddm_dbg"].ap(), DDt["m"][:])
            nc.sync.dma_start(dbg_f["usb_dbg"].ap(), usb[:])
            pm_sb = sbt("pm_sb", (128, 12), F32)
            nc.vector.tensor_copy(pm_sb[:], P["m"][:])
            nc.sync.dma_start(dbg_f["pm_dbg"].ap(), pm_sb[:])

        # ---- output: transpose HmAB columns back to rows, convert, DMA ----
        with tc.tile_pool(name="out_ps", bufs=2, space="PSUM") as ops:
            r0 = 0
            while r0 < T + 1:
                n = min(128, T + 1 - r0)
                ot = ops.tile([128, 152], BF16, tag="ot", name="ot")
                nc.tensor.transpose(ot[0:n, 0:128],
                                    HmAB[0:128, 2 * r0:2 * (r0 + n):2],
                                    Ibf[0:128, 0:128])
                nc.tensor.transpose(ot[0:n, 128:150],
                                    HmAB[0:22, 2 * r0 + 1:2 * (r0 + n):2],
                                    Ibf[0:22, 0:22])
                nc.vector.tensor_copy(OutR[0:n, 0:150], ot[0:n, 0:150])
                nc.sync.dma_start(hr_d.ap()[r0:r0 + n, 0:H], OutR[0:n, 0:150])
                r0 += n

    nc.compile()
    return nc


def _bf(x):
    return np.ascontiguousarray(np.asarray(x, np.float32)).astype(BF)


def prep_shared(E, Wq, Wp, Wr, w, ctx_Wih, ctx_Whh, ctx_bih, ctx_bhh,
                q_Wih, q_Whh, q_bih, q_bhh, m_Wih, m_Whh, m_bih, m_bhh):
    f = {}
    f["Ifp"] = np.eye(128, dtype=np.float32)
    f["Ibf"] = _bf(np.eye(128))
    ir = np.zeros((1, 1024), np.float32)
    ir[0, 1::2] = 1.0
    f["initrow"] = _bf(ir)
    f["onesr"] = _bf(np.ones((1, 512)))
    f["bihm"] = _bf(np.asarray(m_bih, np.float32)[None, :])
    wc = np.zeros((128, 2), np.float32)
    wf = np.asarray(w, np.float32)
    wc[0:128, 0] = wf[0:128]
    wc[0:22, 1] = wf[128:150]
    f["wcol"] = _bf(wc)

    def wih_chunks(pfx, Wih, bih):
        WT = np.asarray(Wih, np.float32).T  # [d, 450]
        f[f"WihT_{pfx}_0"] = _bf(WT[0:128])
        f[f"WihT_{pfx}_1"] = _bf(WT[128:256])
        f[f"WihT_{pfx}_2"] = _bf(np.vstack([WT[256:300],
                                            np.asarray(bih, np.float32)[None, :]]))

    def whh_chunks(pfx, Whh, bhh):
        WT = np.asarray(Whh, np.float32).T  # [150, 450]
        f[f"WhhT_{pfx}_0"] = _bf(WT[0:128])
        f[f"WhhT_{pfx}_1"] = _bf(np.vstack([WT[128:150],
                                            np.asarray(bhh, np.float32)[None, :]]))

    def plain_chunks(pfx, M):
        M = np.asarray(M, np.float32)
        f[f"{pfx}_0"] = _bf(M[0:128])
        f[f"{pfx}_1"] = _bf(M[128:150])

    wih_chunks("q", q_Wih, q_bih)
    wih_chunks("c", ctx_Wih, ctx_bih)
    whh_chunks("q", q_Whh, q_bhh)
    whh_chunks("c", ctx_Whh, ctx_bhh)
    whh_chunks("m", m_Whh, m_bhh)
    m_Wih = np.asarray(m_Wih, np.float32)
    plain_chunks("WcT", m_Wih[:, :H].T)
    plain_chunks("W2T", m_Wih[:, H:].T)
    plain_chunks("Wp", np.asarray(Wp, np.float32))
    plain_chunks("Wr", np.asarray(Wr, np.float32))
    plain_chunks("Wq", np.asarray(Wq, np.float32))
    return f


_NC_CACHE = {}


def kernel(context, query, E, Wq, Wp, Wr, w, ctx_Wih, ctx_Whh, ctx_bih,
           ctx_bhh, q_Wih, q_Whh, q_bih, q_bhh, m_Wih, m_Whh, m_bih, m_bhh,
           _T=None):
    context = np.asarray(context)
    query = np.asarray(query)
    B, T = context.shape
    if _T is not None:
        T = _T
        context = context[:, :T]
    NT = math.ceil(T / 128)
    if T not in _NC_CACHE:
        _NC_CACHE[T] = build(T)
    nc = _NC_CACHE[T]

    shared = prep_shared(E, Wq, Wp, Wr, w, ctx_Wih, ctx_Whh, ctx_bih, ctx_bhh,
                         q_Wih, q_Whh, q_bih, q_bhh, m_Wih, m_Whh, m_bih, m_bhh)
    E_np = np.ascontiguousarray(np.asarray(E, np.float32))
    in_maps = []
    for b in range(B):
        m = dict(shared)
        m["E"] = E_np
        ci = np.zeros((128, NT), np.int32)
        flat = np.asarray(context[b], np.int64).astype(np.int32)
        for g in range(NT):
            n = min(128, T - 128 * g)
            ci[0:n, g] = flat[128 * g:128 * g + n]
        m["ctx_idx"] = ci
        m["q_idx"] = np.asarray(query[b], np.int64).astype(np.int32)[:, None]
        in_maps.append(m)

    res = run_bass_kernel_spmd(nc, in_maps, core_ids=list(range(B)))
    out = np.stack([r["hr"] for r in res.results], axis=0)
    return out.astype(np.float32)


# revision 3
# speedup vs baseline: 1.0111x; 1.0111x over previous
"""MatchLSTM Trainium2 kernel v2: data-parallel over batch (8 cores, 1 elem each).

Column-form recurrences: hidden state lives as SBUF columns (chunk0 [128,1],
chunk1 [22,1]+bias-lane), every per-step matmul outputs a [<=128,1] PSUM
column (lhsT = weight slice, rhs = hidden/embedding column), and the gate
elementwise tail uses fused tensor_scalar ops on per-partition columns.
No per-step transposes. The ctx-GRU, q-GRU and match loops are independent
dep-chains that the Tile scheduler overlaps.
"""
import math
from contextlib import ExitStack

import numpy as np
import ml_dtypes

import concourse.bacc as bacc
import concourse.bass as bass
import concourse.mybir as mybir
import concourse.tile as tile
from concourse.bass_utils import run_bass_kernel_spmd

F32 = mybir.dt.float32
BF16 = mybir.dt.bfloat16
I32 = mybir.dt.int32
AF = mybir.ActivationFunctionType
OP = mybir.AluOpType
BF = ml_dtypes.bfloat16

H = 150
D = 300
J = 64
V = 100000

# gate-column layout inside the per-GRU psum tile:
#   col 0: r0 (gates   0:128), col 1: z0 (150:278),
#   col 2: r1 (128:150),       col 3: z1 (278:300),
#   col 4: hn0 (300:428),      col 5: hn1 (428:450),
#   col 6: xn0 (300:428 x-part), col 7: xn1 (428:450 x-part)
RZ_COLS = [(0, 0, 128), (1, 150, 278), (2, 128, 150), (3, 278, 300)]
HN_COLS = [(4, 300, 428), (5, 428, 450)]
XN_COLS = [(6, 300, 428), (7, 428, 450)]


def build(T=400, dbg=False):
    NT = math.ceil(T / 128)
    tsz = [min(128, T - 128 * g) for g in range(NT)]

    nc = bacc.Bacc("TRN2", target_bir_lowering=False, debug=False, num_devices=8)

    dram = {}

    def din(name, shape, dt):
        dram[name] = nc.dram_tensor(name, list(shape), dt, kind="ExternalInput")
        return dram[name]

    E_d = din("E", [V, D], F32)
    din("ctx_idx", [128, NT], I32)
    din("q_idx", [J, 1], I32)
    din("Ifp", [128, 128], F32)
    din("Ibf", [128, 128], BF16)
    din("initrow", [1, 1024], BF16)  # 0,1,0,1,... for aug bias lanes
    din("onesr", [1, 512], BF16)
    din("bihm", [1, 450], BF16)
    din("wcol", [128, 2], BF16)
    wnames = []
    for g in ("q", "c"):
        wnames += [(f"WihT_{g}_0", (128, 450)), (f"WihT_{g}_1", (128, 450)),
                   (f"WihT_{g}_2", (45, 450))]
    for g in ("q", "c", "m"):
        wnames += [(f"WhhT_{g}_0", (128, 450)), (f"WhhT_{g}_1", (23, 450))]
    wnames += [("WcT_0", (128, 450)), ("WcT_1", (22, 450)),
               ("W2T_0", (128, 450)), ("W2T_1", (22, 450)),
               ("Wp_0", (128, H)), ("Wp_1", (22, H)),
               ("Wr_0", (128, H)), ("Wr_1", (22, H)),
               ("Wq_0", (128, H)), ("Wq_1", (22, H))]
    for n, s in wnames:
        din(n, s, BF16)
    hr_d = nc.dram_tensor("hr", [T + 1, H], F32, kind="ExternalOutput")
    if dbg:
        dbg_d = {n: nc.dram_tensor(n, s, BF16, kind="ExternalOutput")
                 for n, s in (("hq_dbg", [128, 2 * (J + 1)]),
                              ("hc_dbg", [128, 2 * (T + 1)]),
                              ("hm_dbg", [128, 2 * (T + 1)]),
                              ("ecT0_dbg", [128, T]),
                              ("gt0_dbg", [128, J]),
                              ("gt1_dbg", [22, J]),
                              ("hqw2_dbg", [J + 1, 450]),
                              ("whqT0_dbg", [128, J]),
                              ("attn_dbg", [J + 1, 1]))}
        dbg_f = {n: nc.dram_tensor(n, s, F32, kind="ExternalOutput")
                 for n, s in (("sm_dbg", [128, 4]), ("xnm_dbg", [128, 2]),
                              ("nnm_dbg", [128, 2]),
                              ("usb_dbg", [128, 2]), ("pm_dbg", [128, 12]))}

    with tile.TileContext(nc) as tc, ExitStack() as st:
        sb = st.enter_context(tc.tile_pool(name="sb", bufs=1))

        def sbt(name, shape, dt):
            return sb.tile(list(shape), dt, tag=name, name=name)

        W = {n: sbt(n, s, BF16) for n, s in wnames}
        Ifp = sbt("Ifp", (128, 128), F32)
        Ibf = sbt("Ibf", (128, 128), BF16)
        bihm = sbt("bihm", (1, 450), BF16)
        wcol = sbt("wcol", (128, 2), BF16)
        cidx = sbt("cidx", (128, NT), I32)
        qidx = sbt("qidx", (J, 1), I32)
        ec = [sbt(f"ec{g}", (128, D), F32) for g in range(NT)]
        eq = sbt("eq", (J, D), F32)
        ecT = [sbt("ecT0", (128, T), BF16), sbt("ecT1", (128, T), BF16),
               sbt("ecT2", (45, T), BF16)]
        eqT = [sbt("eqT0", (128, J), BF16), sbt("eqT1", (128, J), BF16),
               sbt("eqT2", (45, J), BF16)]
        HqAB = sbt("HqAB", (128, 2 * (J + 1)), BF16)
        HcAB = sbt("HcAB", (128, 2 * (T + 1)), BF16)
        HmAB = sbt("HmAB", (128, 2 * (T + 1)), BF16)
        whqT0 = sbt("whqT0", (128, J), BF16)
        whqT1 = sbt("whqT1", (22, J), BF16)
        HqW2 = sbt("HqW2", (J + 1, 450), BF16)
        attn_aug = sbt("attn_aug", (J + 1, 1), BF16)
        GT0 = sbt("GT0", (128, 2 * J), BF16)
        GT1 = sbt("GT1", (22, J), BF16)
        usb = sbt("usb", (128, 2), F32)
        S = {g: sbt(f"S_{g}", (128, 4), F32) for g in ("q", "c", "m")}
        XN = {g: sbt(f"XN_{g}", (128, 2), F32) for g in ("q", "c", "m")}
        NNt = {g: sbt(f"NN_{g}", (128, 2), F32) for g in ("q", "c", "m")}
        DDt = {g: sbt(f"DD_{g}", (128, 2), F32) for g in ("q", "c", "m")}
        OutR = sbt("OutR", (128, 152), F32)

        # ---- load inputs ----
        for n, _ in wnames:
            nc.sync.dma_start(W[n][:], dram[n].ap())
        nc.sync.dma_start(Ifp[:], dram["Ifp"].ap())
        nc.sync.dma_start(Ibf[:], dram["Ibf"].ap())
        nc.sync.dma_start(bihm[:], dram["bihm"].ap())
        nc.sync.dma_start(wcol[:], dram["wcol"].ap())
        nc.sync.dma_start(cidx[:], dram["ctx_idx"].ap())
        nc.sync.dma_start(qidx[:], dram["q_idx"].ap())

        # ---- init state: h0 = 0; aug bias lane (row 22 of odd cols) = 1.0 ----
        for hab, ncols in ((HqAB, 2 * (J + 1)), (HcAB, 2 * (T + 1)),
                           (HmAB, 2 * (T + 1))):
            nc.vector.memset(hab[:, 0:2], 0.0)
            nc.sync.dma_start(hab[22:23, 0:ncols],
                             dram["initrow"].ap()[0:1, 0:ncols])
        nc.vector.memset(attn_aug[0:J + 1, 0:1], 0.0)
        nc.vector.memset(attn_aug[J:J + 1, 0:1], 1.0)
        nc.sync.dma_start(ecT[2][44:45, 0:T], dram["onesr"].ap()[0:1, 0:T])
        nc.sync.dma_start(eqT[2][44:45, 0:J], dram["onesr"].ap()[0:1, 0:J])
        nc.sync.dma_start(HqW2[J:J + 1, :], dram["bihm"].ap())

        # ---- embedding gathers ----
        for g in range(NT):
            nc.gpsimd.indirect_dma_start(
                out=ec[g][:], out_offset=None, in_=E_d.ap(),
                in_offset=bass.IndirectOffsetOnAxis(ap=cidx[:, g:g + 1], axis=0))
        nc.gpsimd.indirect_dma_start(
            out=eq[:], out_offset=None, in_=E_d.ap(),
            in_offset=bass.IndirectOffsetOnAxis(ap=qidx[:, 0:1], axis=0))

        dch = [(0, 128), (128, 128), (256, 44)]

        # ---- embedding transposes: ec/eq -> ecT/eqT (bf16 columns) ----
        with tc.tile_pool(name="pre_ps", bufs=2, space="PSUM") as pps:
            for g in range(NT):
                toff = 128 * g
                for k, (doff, dsz) in enumerate(dch):
                    tp = pps.tile([128, 128], F32, tag="tp", name="tp")
                    nc.tensor.transpose(tp[0:dsz, 0:tsz[g]],
                                        ec[g][0:tsz[g], doff:doff + dsz],
                                        Ifp[0:tsz[g], 0:tsz[g]])
                    nc.vector.tensor_copy(ecT[k][0:dsz, toff:toff + tsz[g]],
                                          tp[0:dsz, 0:tsz[g]])
            for k, (doff, dsz) in enumerate(dch):
                tp = pps.tile([128, 128], F32, tag="tp", name="tp")
                nc.tensor.transpose(tp[0:dsz, 0:J], eq[0:J, doff:doff + dsz],
                                    Ifp[0:J, 0:J])
                nc.vector.tensor_copy(eqT[k][0:dsz, 0:J], tp[0:dsz, 0:J])

        # ---- persistent psum pools (one bank-sized tile per GRU chain) ----
        # columns: 0:8 gate psum P, 8:10 narg, 10:12 nn, 12:14 u, 14:15 attn
        psA = st.enter_context(tc.tile_pool(name="psA", bufs=1, space="PSUM"))
        P = {g: psA.tile([128, 12], F32, tag=f"PS_{g}", name=f"PS_{g}")
             for g in ("q", "c", "m")}
        PG = psA.tile([128, 2 * J], F32, tag="PG", name="PG")
        CA = 10            # attn column offset in the match PS tile
        CG0, CG1 = 0, J    # G^T pre-activation chunks in PG

        def gru_tail(g, Pt, hprev_pair, hout0, hout1):
            """Gate elementwise tail shared by all three GRUs.

            hprev_pair: [128,2] AP of the previous hidden column pair;
            hout*: APs for the new hidden column chunks."""
            Sg, XNg, NNg, DDg = S[g], XN[g], NNt[g], DDt[g]
            # xn (x-part of n gate) psum -> sbuf (one [128,2] copy; junk rows ok)
            nc.vector.tensor_copy(XNg[0:128, 0:2], Pt[0:128, 6:8])
            # sigmoid over r0,z0,r1,z1 in one shot (junk rows of cols 2:4 unused)
            nc.scalar.activation(Sg[0:128, 0:4], Pt[0:128, 0:4], AF.Sigmoid)
            # n = tanh(r * hn + xn) fused into one activation per chunk
            nc.scalar.activation(NNg[0:128, 0:1], Pt[0:128, 4:5], AF.Tanh,
                                 bias=XNg[0:128, 0:1], scale=Sg[0:128, 0:1])
            nc.scalar.activation(NNg[0:22, 1:2], Pt[0:22, 5:6], AF.Tanh,
                                 bias=XNg[0:22, 1:2], scale=Sg[0:22, 2:3])
            # dd = h - n  (both chunks in one op; junk rows unused)
            nc.gpsimd.tensor_tensor(out=DDg[0:128, 0:2], in0=hprev_pair,
                                    in1=NNg[0:128, 0:2], op=OP.subtract)
            # h2 = dd * z + n
            nc.gpsimd.tensor_scalar(
                out=hout0, in0=DDg[0:128, 0:1], scalar1=Sg[0:128, 1:2],
                scalar2=NNg[0:128, 0:1], op0=OP.mult, op1=OP.add)
            nc.vector.tensor_scalar(
                out=hout1, in0=DDg[0:22, 1:2], scalar1=Sg[0:22, 3:4],
                scalar2=NNg[0:22, 1:2], op0=OP.mult, op1=OP.add)

        def enc_step(g, t, HAB, xT, xsz):
            """One encoder GRU step: x-projection + h-projection column mms,
            then the gate tail. xT = [ecT|eqT], xsz = per-chunk K sizes."""
            Pt = P[g]
            w0, w1 = W[f"WhhT_{g}_0"], W[f"WhhT_{g}_1"]
            x0, x1, x2 = (W[f"WihT_{g}_0"], W[f"WihT_{g}_1"],
                          W[f"WihT_{g}_2"])
            h0 = HAB[:, 2 * t:2 * t + 1]
            h1 = HAB[0:23, 2 * t + 1:2 * t + 2]
            # per-column consecutive accumulation groups: x k-chunks + h-proj
            for col, m0, m1 in RZ_COLS + XN_COLS:
                msz = m1 - m0
                for k, (xw, ksz) in enumerate(((x0, xsz[0]), (x1, xsz[1]),
                                               (x2, xsz[2]))):
                    last = (k == 2) and col in (6, 7)
                    nc.tensor.matmul(Pt[0:msz, col:col + 1],
                                     xw[0:ksz, m0:m1],
                                     xT[k][0:ksz, t:t + 1],
                                     start=(k == 0), stop=last)
                if col not in (6, 7):
                    nc.tensor.matmul(Pt[0:msz, col:col + 1], w0[:, m0:m1], h0,
                                     start=False, stop=False)
                    nc.tensor.matmul(Pt[0:msz, col:col + 1], w1[0:23, m0:m1],
                                     h1, start=False, stop=True)
            for col, m0, m1 in HN_COLS:
                msz = m1 - m0
                nc.tensor.matmul(Pt[0:msz, col:col + 1], w0[:, m0:m1], h0,
                                 start=True, stop=False)
                nc.tensor.matmul(Pt[0:msz, col:col + 1], w1[0:23, m0:m1], h1,
                                 start=False, stop=True)
            gru_tail(g, Pt, HAB[0:128, 2 * t:2 * t + 2],
                     HAB[0:128, 2 * t + 2:2 * t + 3],
                     HAB[0:22, 2 * t + 3:2 * t + 4])

        def match_step(t):
            """One match step: u = Wr@hm + Wp@hc; G = tanh(whqT + u);
            attn = G^T w; gates = Wc@hc + Whh@hm + HqW2^T@[attn;1]."""
            Pt = P["m"]
            hm0 = HmAB[:, 2 * t:2 * t + 1]
            hm1 = HmAB[0:23, 2 * t + 1:2 * t + 2]
            hm1s = HmAB[0:22, 2 * t + 1:2 * t + 2]
            hc0 = HcAB[:, 2 * t + 2:2 * t + 3]
            hc1 = HcAB[0:22, 2 * t + 3:2 * t + 4]
            # G^T pre-activation in PSUM: whqT + (Wr hm + Wp hc) (x) ones
            hm0b = hm0.broadcast_to([128, J])
            hm1b = hm1s.broadcast_to([22, J])
            hc0b = hc0.broadcast_to([128, J])
            hc1b = hc1.broadcast_to([22, J])
            for (gc, m0, m1, idn, wq) in ((CG0, 0, 128, 128, whqT0),
                                          (CG1, 128, 150, 22, whqT1)):
                msz = m1 - m0
                nc.tensor.matmul(PG[0:msz, gc:gc + J], Ibf[0:idn, 0:msz],
                                 wq[0:idn, 0:J], start=True, stop=False)
                nc.tensor.matmul(PG[0:msz, gc:gc + J], W["Wr_0"][:, m0:m1],
                                 hm0b, start=False, stop=False)
                nc.tensor.matmul(PG[0:msz, gc:gc + J], W["Wr_1"][0:22, m0:m1],
                                 hm1b, start=False, stop=False)
                nc.tensor.matmul(PG[0:msz, gc:gc + J], W["Wp_0"][:, m0:m1],
                                 hc0b, start=False, stop=False)
                nc.tensor.matmul(PG[0:msz, gc:gc + J], W["Wp_1"][0:22, m0:m1],
                                 hc1b, start=False, stop=True)
            nc.scalar.activation(GT0[0:128, 0:2 * J], PG[0:128, 0:2 * J],
                                 AF.Tanh)
            # attn column = GT^T w
            nc.tensor.matmul(Pt[0:J, CA:CA + 1], GT0[0:128, 0:J],
                             wcol[0:128, 0:1], start=True, stop=False)
            nc.tensor.matmul(Pt[0:J, CA:CA + 1], GT0[0:22, J:2 * J],
                             wcol[0:22, 1:2], start=False, stop=True)
            nc.vector.tensor_copy(attn_aug[0:J, 0:1], Pt[0:J, CA:CA + 1])
            # gate columns: zx (Wc@hc) + Whh@hm + HqW2^T@[attn;1]
            # each column's accumulation group is emitted consecutively
            for col, m0, m1 in RZ_COLS + XN_COLS:
                msz = m1 - m0
                nc.tensor.matmul(Pt[0:msz, col:col + 1], W["WcT_0"][:, m0:m1],
                                 hc0, start=True, stop=False)
                nc.tensor.matmul(Pt[0:msz, col:col + 1],
                                 W["WcT_1"][0:22, m0:m1], hc1,
                                 start=False, stop=False)
                if col not in (6, 7):
                    nc.tensor.matmul(Pt[0:msz, col:col + 1],
                                     W["WhhT_m_0"][:, m0:m1], hm0,
                                     start=False, stop=False)
                    nc.tensor.matmul(Pt[0:msz, col:col + 1],
                                     W["WhhT_m_1"][0:23, m0:m1], hm1,
                                     start=False, stop=False)
                nc.tensor.matmul(Pt[0:msz, col:col + 1],
                                 HqW2[0:J + 1, m0:m1], attn_aug[0:J + 1, 0:1],
                                 start=False, stop=True)
            for col, m0, m1 in HN_COLS:
                msz = m1 - m0
                nc.tensor.matmul(Pt[0:msz, col:col + 1],
                                 W["WhhT_m_0"][:, m0:m1], hm0,
                                 start=True, stop=False)
                nc.tensor.matmul(Pt[0:msz, col:col + 1],
                                 W["WhhT_m_1"][0:23, m0:m1], hm1,
                                 start=False, stop=True)
            gru_tail("m", Pt, HmAB[0:128, 2 * t:2 * t + 2],
                     HmAB[0:128, 2 * t + 2:2 * t + 3],
                     HmAB[0:22, 2 * t + 3:2 * t + 4])

        # ---- q-GRU ----
        for j in range(J):
            enc_step("q", j, HqAB, eqT, (128, 128, 45))
        # ---- whqT + HqW2 prep ----
        hq_c0 = HqAB[0:128, 2:2 * (J + 1):2]
        hq_c1 = HqAB[0:22, 3:2 * (J + 1):2]
        with tc.tile_pool(name="prep_ps", bufs=1, space="PSUM") as qps:
            wq_ps = qps.tile([128, J], F32, tag="wq", name="wq")
            hw_ps = qps.tile([J, 450], F32, tag="hw", name="hw")
            for ci, (m0, m1) in enumerate(((0, 128), (128, 150))):
                msz = m1 - m0
                nc.tensor.matmul(wq_ps[0:msz, 0:J], W["Wq_0"][:, m0:m1], hq_c0,
                                 start=True, stop=False)
                nc.tensor.matmul(wq_ps[0:msz, 0:J], W["Wq_1"][0:22, m0:m1],
                                 hq_c1, start=False, stop=True)
                dst = whqT0 if ci == 0 else whqT1
                nc.vector.tensor_copy(dst[0:msz, 0:J], wq_ps[0:msz, 0:J])
            nc.tensor.matmul(hw_ps[0:J, :], hq_c0, W["W2T_0"][:, :],
                             start=True, stop=False)
            nc.tensor.matmul(hw_ps[0:J, :], hq_c1, W["W2T_1"][0:22, :],
                             start=False, stop=True)
            nc.vector.tensor_copy(HqW2[0:J, :], hw_ps[0:J, :])

        # ---- ctx-GRU + match loop (scheduler overlaps the chains) ----
        for t in range(T):
            enc_step("c", t, HcAB, ecT, (128, 128, 45))
            match_step(t)

        if dbg:
            nc.sync.dma_start(dbg_d["hq_dbg"].ap(), HqAB[:])
            nc.sync.dma_start(dbg_d["hc_dbg"].ap(), HcAB[:])
            nc.sync.dma_start(dbg_d["hm_dbg"].ap(), HmAB[:])
            nc.sync.dma_start(dbg_d["ecT0_dbg"].ap(), ecT[0][:])
            nc.sync.dma_start(dbg_d["gt0_dbg"].ap(), GT0[:])
            nc.sync.dma_start(dbg_d["gt1_dbg"].ap(), GT1[:])
            nc.sync.dma_start(dbg_d["hqw2_dbg"].ap(), HqW2[:])
            nc.sync.dma_start(dbg_d["whqT0_dbg"].ap(), whqT0[:])
            nc.sync.dma_start(dbg_d["attn_dbg"].ap(), attn_aug[:])
            nc.sync.dma_start(dbg_f["sm_dbg"].ap(), S["m"][:])
            nc.sync.dma_start(dbg_f["xnm_dbg"].ap(), XN["m"][:])
            nc.sync.dma_start(dbg_f["nnm_dbg"].ap(), NNt["m"][:])
            nc.sync.dma_start(dbg_f["usb_dbg"].ap(), usb[:])
            pm_sb = sbt("pm_sb", (128, 12), F32)
            nc.vector.tensor_copy(pm_sb[:], P["m"][:])
            nc.sync.dma_start(dbg_f["pm_dbg"].ap(), pm_sb[:])

        # ---- output: transpose HmAB columns back to rows, convert, DMA ----
        with tc.tile_pool(name="out_ps", bufs=2, space="PSUM") as ops:
            r0 = 0
            while r0 < T + 1:
                n = min(128, T + 1 - r0)
                ot = ops.tile([128, 152], BF16, tag="ot", name="ot")
                nc.tensor.transpose(ot[0:n, 0:128],
                                    HmAB[0:128, 2 * r0:2 * (r0 + n):2],
                                    Ibf[0:128, 0:128])
                nc.tensor.transpose(ot[0:n, 128:150],
                                    HmAB[0:22, 2 * r0 + 1:2 * (r0 + n):2],
                                    Ibf[0:22, 0:22])
                nc.vector.tensor_copy(OutR[0:n, 0:150], ot[0:n, 0:150])
                nc.sync.dma_start(hr_d.ap()[r0:r0 + n, 0:H], OutR[0:n, 0:150])
                r0 += n

    nc.compile()
    return nc


def _bf(x):
    return np.ascontiguousarray(np.asarray(x, np.float32)).astype(BF)


def prep_shared(E, Wq, Wp, Wr, w, ctx_Wih, ctx_Whh, ctx_bih, ctx_bhh,
                q_Wih, q_Whh, q_bih, q_bhh, m_Wih, m_Whh, m_bih, m_bhh):
    f = {}
    f["Ifp"] = np.eye(128, dtype=np.float32)
    f["Ibf"] = _bf(np.eye(128))
    ir = np.zeros((1, 1024), np.float32)
    ir[0, 1::2] = 1.0
    f["initrow"] = _bf(ir)
    f["onesr"] = _bf(np.ones((1, 512)))
    f["bihm"] = _bf(np.asarray(m_bih, np.float32)[None, :])
    wc = np.zeros((128, 2), np.float32)
    wf = np.asarray(w, np.float32)
    wc[0:128, 0] = wf[0:128]
    wc[0:22, 1] = wf[128:150]
    f["wcol"] = _bf(wc)

    def wih_chunks(pfx, Wih, bih):
        WT = np.asarray(Wih, np.float32).T  # [d, 450]
        f[f"WihT_{pfx}_0"] = _bf(WT[0:128])
        f[f"WihT_{pfx}_1"] = _bf(WT[128:256])
        f[f"WihT_{pfx}_2"] = _bf(np.vstack([WT[256:300],
                                            np.asarray(bih, np.float32)[None, :]]))

    def whh_chunks(pfx, Whh, bhh):
        WT = np.asarray(Whh, np.float32).T  # [150, 450]
        f[f"WhhT_{pfx}_0"] = _bf(WT[0:128])
        f[f"WhhT_{pfx}_1"] = _bf(np.vstack([WT[128:150],
                                            np.asarray(bhh, np.float32)[None, :]]))

    def plain_chunks(pfx, M):
        M = np.asarray(M, np.float32)
        f[f"{pfx}_0"] = _bf(M[0:128])
        f[f"{pfx}_1"] = _bf(M[128:150])

    wih_chunks("q", q_Wih, q_bih)
    wih_chunks("c", ctx_Wih, ctx_bih)
    whh_chunks("q", q_Whh, q_bhh)
    whh_chunks("c", ctx_Whh, ctx_bhh)
    whh_chunks("m", m_Whh, m_bhh)
    m_Wih = np.asarray(m_Wih, np.float32)
    plain_chunks("WcT", m_Wih[:, :H].T)
    plain_chunks("W2T", m_Wih[:, H:].T)
    plain_chunks("Wp", np.asarray(Wp, np.float32))
    plain_chunks("Wr", np.asarray(Wr, np.float32))
    plain_chunks("Wq", np.asarray(Wq, np.float32))
    return f


_NC_CACHE = {}


def kernel(context, query, E, Wq, Wp, Wr, w, ctx_Wih, ctx_Whh, ctx_bih,
           ctx_bhh, q_Wih, q_Whh, q_bih, q_bhh, m_Wih, m_Whh, m_bih, m_bhh,
           _T=None):
    context = np.asarray(context)
    query = np.asarray(query)
    B, T = context.shape
    if _T is not None:
        T = _T
        context = context[:, :T]
    NT = math.ceil(T / 128)
    if T not in _NC_CACHE:
        _NC_CACHE[T] = build(T)
    nc = _NC_CACHE[T]

    shared = prep_shared(E, Wq, Wp, Wr, w, ctx_Wih, ctx_Whh, ctx_bih, ctx_bhh,
                         q_Wih, q_Whh, q_bih, q_bhh, m_Wih, m_Whh, m_bih, m_bhh)
    E_np = np.ascontiguousarray(np.asarray(E, np.float32))
    in_maps = []
    for b in range(B):
        m = dict(shared)
        m["E"] = E_np
        ci = np.zeros((128, NT), np.int32)
        flat = np.asarray(context[b], np.int64).astype(np.int32)
        for g in range(NT):
            n = min(128, T - 128 * g)
            ci[0:n, g] = flat[128 * g:128 * g + n]
        m["ctx_idx"] = ci
        m["q_idx"] = np.asarray(query[b], np.int64).astype(np.int32)[:, None]
        in_maps.append(m)

    res = run_bass_kernel_spmd(nc, in_maps, core_ids=list(range(B)))
    out = np.stack([r["hr"] for r in res.results], axis=0)
    return out.astype(np.float32)
